# revision 1
# baseline (speedup 1.0000x reference)
"""Data-parallel Trainium2 attention kernel (B=8 sharded over 8 NeuronCores).

Per core (one batch element):
  qkv projections (bf16 matmuls, biases via K=1 ones-matmuls, softmax scale
  folded into Wq/bq on host)
  scores computed transposed [k, q] via 4-head row-tiled matmuls (dh=32)
  bias handled as exp(s+rel) = exp(s) * exp(rel) with host-precomputed
  exp(rel_pos).T in bf16; ACT exps scores from PSUM, DVE multiplies
  PV + softmax denominators via col-tiled matmuls (M=32 v-cols + M=1 ones)
  normalization via reciprocal + partition-broadcast DMA, then output proj
"""

import sys

sys.path.insert(0, "/opt/trn_rl_repo")

import numpy as np
import ml_dtypes

B, N, C, H, DH = 8, 1024, 768, 24, 32
HG = H // 4  # 6 groups of 4 heads
SCALE = DH ** -0.5
BF16 = ml_dtypes.bfloat16

_CACHE = {}


def _build():
    if "nc" in _CACHE:
        return _CACHE["nc"]
    from contextlib import ExitStack
    import concourse.mybir as mybir
    import concourse.tile as tile
    from concourse import bacc

    nc = bacc.Bacc("TRN2")
    bf, f32 = mybir.dt.bfloat16, mybir.dt.float32
    Exp = mybir.ActivationFunctionType.Exp

    xT_d = nc.declare_dram_parameter("xT", [C, N], bf, isOutput=False)
    wq_d = nc.declare_dram_parameter("wq", [C, C], bf, isOutput=False)
    wk_d = nc.declare_dram_parameter("wk", [C, C], bf, isOutput=False)
    wv_d = nc.declare_dram_parameter("wv", [C, C], bf, isOutput=False)
    bq_d = nc.declare_dram_parameter("bq", [1, C], bf, isOutput=False)
    bk_d = nc.declare_dram_parameter("bk", [1, C], bf, isOutput=False)
    bv_d = nc.declare_dram_parameter("bv", [1, C], bf, isOutput=False)
    eb_d = nc.declare_dram_parameter("expb", [HG, N, 4, N], bf, isOutput=False)
    wp_d = nc.declare_dram_parameter("wpj", [C, C], bf, isOutput=False)
    bp_d = nc.declare_dram_parameter("bpj", [1, C], bf, isOutput=False)
    out_d = nc.declare_dram_parameter("out", [N, C], f32, isOutput=True)
    scr_d = nc.dram_tensor("recscr", [128, 12, 512], bf)

    with tile.TileContext(nc) as tc, ExitStack() as ctx:
        ctx.enter_context(nc.allow_low_precision(
            reason="bf16 compute intentional; rel_err budget 2e-2"))
        const = ctx.enter_context(tc.tile_pool(name="const", bufs=1))
        big = ctx.enter_context(tc.tile_pool(name="big", bufs=1))
        prb = ctx.enter_context(tc.tile_pool(name="prb", bufs=2))
        stage = ctx.enter_context(tc.tile_pool(name="stage", bufs=2))
        exps_p = ctx.enter_context(tc.tile_pool(name="exps", bufs=3))
        norm = ctx.enter_context(tc.tile_pool(name="norm", bufs=2))
        ypool = ctx.enter_context(tc.tile_pool(name="ypool", bufs=3))
        psA = ctx.enter_context(tc.tile_pool(name="psA", bufs=3, space="PSUM"))
        psB = ctx.enter_context(tc.tile_pool(name="psB", bufs=2, space="PSUM"))

        # ---- constant loads
        xT = const.tile([128, 6, N], bf)
        nc.sync.dma_start(out=xT, in_=xT_d.rearrange("(s p) n -> p s n", p=128))
        wq = const.tile([128, 6, C], bf)
        nc.sync.dma_start(out=wq, in_=wq_d.rearrange("(s p) m -> p s m", p=128))
        wk = const.tile([128, 6, C], bf)
        nc.sync.dma_start(out=wk, in_=wk_d.rearrange("(s p) m -> p s m", p=128))
        wv = const.tile([128, 6, C], bf)
        nc.sync.dma_start(out=wv, in_=wv_d.rearrange("(s p) m -> p s m", p=128))
        wp = const.tile([128, 6, C], bf)
        nc.sync.dma_start(out=wp, in_=wp_d.rearrange("(s p) m -> p s m", p=128))
        bq = const.tile([1, C], bf)
        nc.sync.dma_start(out=bq, in_=bq_d[:, :])
        bk = const.tile([1, C], bf)
        nc.sync.dma_start(out=bk, in_=bk_d[:, :])
        bv = const.tile([1, C], bf)
        nc.sync.dma_start(out=bv, in_=bv_d[:, :])
        bp = const.tile([1, C], bf)
        nc.sync.dma_start(out=bp, in_=bp_d[:, :])
        ones512 = const.tile([1, 512], bf)
        nc.vector.memset(ones512, 1.0)
        ones128 = const.tile([128, 1], bf)
        nc.vector.memset(ones128, 1.0)

        # ---- persistent intermediates
        qT = big.tile([128, 6, N], bf)      # q*scale+bq, [32h+d -> (p,s)], n
        kT = big.tile([128, 6, N], bf)
        v = big.tile([128, 8, H, DH], bf)   # [token%128, token//128, h, d]
        outT = big.tile([128, 6, N], bf)    # unnorm attn out.T [32h+d, n]

        # ---- qkv projections.  Emission order: q/k for groups 0-1 first
        # (lets attention g=0 start early), then v (needed by PV phase B),
        # then the remaining q/k groups.
        def emit_qk_group(j):
            for (wt, bt, dstT) in ((wq, bq, qT), (wk, bk, kT)):
                for t in range(2):
                    ps = psA.tile([128, 2, 512], f32, tag="ps")
                    pq = ps[:, 0, :]
                    for s in range(6):
                        nc.tensor.matmul(
                            pq,
                            lhsT=wt[:, s, 128 * j:128 * (j + 1)],
                            rhs=xT[:, s, 512 * t:512 * (t + 1)],
                            start=(s == 0), stop=False)
                    nc.tensor.matmul(
                        pq, lhsT=bt[:, 128 * j:128 * (j + 1)],
                        rhs=ones512[:, :], start=False, stop=True)
                    nc.vector.tensor_copy(
                        out=dstT[:, j, 512 * t:512 * (t + 1)], in_=pq)

        def emit_v():
            for i in range(8):
                for (f0, fw) in ((0, 512), (512, 256)):
                    ps = psA.tile([128, 2, 512], f32, tag="ps")
                    pv_ = ps[:, 0, :fw]
                    for s in range(6):
                        nc.tensor.matmul(
                            pv_,
                            lhsT=xT[:, s, 128 * i:128 * (i + 1)],
                            rhs=wv[:, s, f0:f0 + fw],
                            start=(s == 0), stop=False)
                    nc.tensor.matmul(
                        pv_, lhsT=ones512[:, :128], rhs=bv[:, f0:f0 + fw],
                        start=False, stop=True)
                    nc.vector.tensor_copy(
                        out=v[:, i, f0 // DH:(f0 + fw) // DH, :],
                        in_=pv_.rearrange("p (h d) -> p h d", d=DH))

        emit_qk_group(0)
        emit_qk_group(1)
        emit_v()
        for j in range(2, 6):
            emit_qk_group(j)

        # ---- attention
        for g in range(6):
            for qt in range(2):
                qs = slice(512 * qt, 512 * (qt + 1))
                probs = prb.tile([128, 4, 8, 512], bf, tag="probs")
                # phase A: scoresT = kT.T@qT (4-head row-tiled), exp, *expb
                for kt in range(8):
                    sc0 = psA.tile([128, 2, 512], f32, tag="ps")
                    sc1 = psA.tile([128, 2, 512], f32, tag="ps")
                    for i in range(4):
                        sct = (sc0 if i < 2 else sc1)[:, i % 2, :]
                        nc.tensor.matmul(
                            sct,
                            lhsT=kT[32 * i:32 * (i + 1), g, 128 * kt:128 * (kt + 1)],
                            rhs=qT[32 * i:32 * (i + 1), g, qs],
                            start=True, stop=True, tile_position=(32 * i, 0))
                    eb = stage.tile([128, 4, 512], bf, tag="eb")
                    nc.sync.dma_start(
                        out=eb, in_=eb_d[g, 128 * kt:128 * (kt + 1), :, qs])
                    ex = exps_p.tile([128, 4, 512], bf, tag="ex")
                    nc.scalar.activation(out=ex[:, 0:2, :], in_=sc0, func=Exp)
                    nc.scalar.activation(out=ex[:, 2:4, :], in_=sc1, func=Exp)
                    nc.vector.tensor_mul(
                        out=probs[:, 0:2, kt, :], in0=ex[:, 0:2, :],
                        in1=eb[:, 0:2, :])
                    nc.vector.tensor_mul(
                        out=probs[:, 2:4, kt, :], in0=ex[:, 2:4, :],
                        in1=eb[:, 2:4, :])
                # phase B: outT/denoms via col-tiled PV (M=32 + M=1 ones)
                pv = psB.tile([128, 512], f32, tag="pv")
                dn = psB.tile([128, 512], f32, tag="pv")
                nc.vector.memset(dn, 1.0)
                for kt in range(8):
                    for hl in range(4):
                        nc.tensor.matmul(
                            pv[32 * hl:32 * (hl + 1), :],
                            lhsT=v[:, kt, 4 * g + hl, :],
                            rhs=probs[:, hl, kt, :],
                            start=(kt == 0), stop=(kt == 7),
                            tile_position=(0, 32 * hl),
                            skip_group_check=True)
                        nc.tensor.matmul(
                            dn[32 * hl:32 * hl + 1, :],
                            lhsT=ones128[:, :],
                            rhs=probs[:, hl, kt, :],
                            start=(kt == 0), stop=(kt == 7),
                            tile_position=(0, 32 * hl),
                            skip_group_check=True)
                nc.vector.tensor_copy(out=outT[:, g, qs], in_=pv)
                # inline normalization: recip -> DRAM -> partition-broadcast
                slot = 2 * g + qt
                dtile = norm.tile([128, 512], bf, tag="den")
                rtile = norm.tile([128, 512], bf, tag="rb")
                nc.vector.tensor_copy(out=dtile, in_=dn)
                nc.vector.reciprocal(out=dtile, in_=dtile)
                nc.sync.dma_start(out=scr_d[:, slot, :], in_=dtile)
                for hl in range(4):
                    nc.sync.dma_start(
                        out=rtile[32 * hl:32 * (hl + 1), :],
                        in_=scr_d[32 * hl:32 * hl + 1, slot, :].to_broadcast(
                            (32, 512)))
                nc.vector.tensor_mul(
                    out=outT[:, g, qs], in0=outT[:, g, qs], in1=rtile)

        # ---- output projection
        for i in range(8):
            ytile = ypool.tile([128, C], f32, tag="y")
            for (f0, fw) in ((0, 512), (512, 256)):
                ps = psA.tile([128, 2, 512], f32, tag="ps")
                py = ps[:, 0, :fw]
                for s in range(6):
                    nc.tensor.matmul(
                        py,
                        lhsT=outT[:, s, 128 * i:128 * (i + 1)],
                        rhs=wp[:, s, f0:f0 + fw],
                        start=(s == 0), stop=False)
                nc.tensor.matmul(
                    py, lhsT=ones512[:, :128], rhs=bp[:, f0:f0 + fw],
                    start=False, stop=True)
                nc.vector.tensor_copy(out=ytile[:, f0:f0 + fw], in_=py)
            nc.sync.dma_start(out=out_d[128 * i:128 * (i + 1), :], in_=ytile)

    nc.finalize()
    _CACHE["nc"] = nc
    return nc


def _prep_shared(shared_rel_pos, Wqkv, bqkv, Wproj, bproj):
    """Host-side weight rearrangement shared by all cores (float32 in)."""
    w3 = np.asarray(Wqkv, np.float32).reshape(H, 3, DH, C)
    wq_t = (w3[:, 0] * SCALE).transpose(2, 0, 1).reshape(C, C)
    wk_t = w3[:, 1].transpose(2, 0, 1).reshape(C, C)
    wv_t = w3[:, 2].transpose(2, 0, 1).reshape(C, C)
    b3 = np.asarray(bqkv, np.float32).reshape(H, 3, DH)
    bq_a = (b3[:, 0] * SCALE).reshape(1, C)
    bk_a = b3[:, 1].reshape(1, C)
    bv_a = b3[:, 2].reshape(1, C)
    # exp(rel)^T grouped: [g, k, hl, q]
    expb = np.exp(np.asarray(shared_rel_pos, np.float32))
    expb = expb.transpose(0, 2, 1).reshape(HG, 4, N, N).transpose(0, 2, 1, 3)
    wp_t = np.asarray(Wproj, np.float32).T.copy()
    bp_a = np.asarray(bproj, np.float32).reshape(1, C)
    return {
        "wq": np.ascontiguousarray(wq_t).astype(BF16),
        "wk": np.ascontiguousarray(wk_t).astype(BF16),
        "wv": np.ascontiguousarray(wv_t).astype(BF16),
        "bq": bq_a.astype(BF16),
        "bk": bk_a.astype(BF16),
        "bv": bv_a.astype(BF16),
        "expb": np.ascontiguousarray(expb).astype(BF16),
        "wpj": wp_t.astype(BF16),
        "bpj": bp_a.astype(BF16),
    }


def _in_maps(x, shared):
    x = np.asarray(x, np.float32)
    maps = []
    for b in range(B):
        m = dict(shared)
        m["xT"] = np.ascontiguousarray(x[b].T).astype(BF16)
        maps.append(m)
    return maps


def kernel(**inputs):
    from concourse.bass_utils import run_bass_kernel_spmd

    nc = _build()
    shared = _prep_shared(
        inputs["shared_rel_pos"], inputs["Wqkv"], inputs["bqkv"],
        inputs["Wproj"], inputs["bproj"])
    maps = _in_maps(inputs["x"], shared)
    res = run_bass_kernel_spmd(nc, maps, core_ids=list(range(B)))
    out = np.stack([np.asarray(res.results[i]["out"], np.float32)
                    for i in range(B)])
    return out



# revision 3
# speedup vs baseline: 1.2747x; 1.2747x over previous
"""Data-parallel Trainium2 attention kernel (B=8 sharded over 8 NeuronCores).

Per core (one batch element):
  qkv projections (bf16 matmuls; q/k biases fused into the PSUM->SBUF copy
  via per-partition tensor_scalar add; softmax scale folded into Wq/bq on
  host; v/proj biases via K=1 ones-matmuls)
  scores computed transposed [k, q] via 16-way tile_position packing
  (4 heads x 4 k-chunks of 32, all concurrent in the 128x128 PE array)
  bias handled as exp(s+rel) = exp(s) * exp(rel) with host-precomputed
  exp(rel_pos).T in bf16; ACT exps scores PSUM->SBUF, DVE multiplies
  in-place (N=2048 per kt tile)
  PV + softmax denominators via col-tiled matmuls (M=32 v-cols + M=1 ones)
  normalization via reciprocal_approx_fast on the PSUM denominators +
  partition-broadcast DMA, then output projection interleaved per qt half
"""

import sys

sys.path.insert(0, "/opt/trn_rl_repo")

import numpy as np
import ml_dtypes

B, N, C, H, DH = 8, 1024, 768, 24, 32
HG = H // 4  # 6 groups of 4 heads
SCALE = DH ** -0.5
BF16 = ml_dtypes.bfloat16

_CACHE = {}


def _build():
    if "nc" in _CACHE:
        return _CACHE["nc"]
    from contextlib import ExitStack
    import concourse.mybir as mybir
    import concourse.tile as tile
    from concourse import bacc

    nc = bacc.Bacc("TRN2")
    bf, f32 = mybir.dt.bfloat16, mybir.dt.float32
    Exp = mybir.ActivationFunctionType.Exp
    Add = mybir.AluOpType.add

    xT_d = nc.declare_dram_parameter("xT", [C, N], bf, isOutput=False)
    wq_d = nc.declare_dram_parameter("wq", [C, C], bf, isOutput=False)
    wk_d = nc.declare_dram_parameter("wk", [C, C], bf, isOutput=False)
    wv_d = nc.declare_dram_parameter("wv", [C, C], bf, isOutput=False)
    bqt_d = nc.declare_dram_parameter("bqt", [128, 6], f32, isOutput=False)
    bkt_d = nc.declare_dram_parameter("bkt", [128, 6], f32, isOutput=False)
    bv_d = nc.declare_dram_parameter("bv", [1, C], bf, isOutput=False)
    eb_d = nc.declare_dram_parameter("expb", [HG, N, 4, N], bf, isOutput=False)
    wp_d = nc.declare_dram_parameter("wpj", [C, C], bf, isOutput=False)
    bp_d = nc.declare_dram_parameter("bpj", [1, C], bf, isOutput=False)
    out_d = nc.declare_dram_parameter("out", [N, C], f32, isOutput=True)
    scr_d = nc.dram_tensor("recscr", [128, 12, 512], bf)

    with tile.TileContext(nc) as tc, ExitStack() as ctx:
        ctx.enter_context(nc.allow_low_precision(
            reason="bf16 compute intentional; rel_err budget 2e-2"))
        const = ctx.enter_context(tc.tile_pool(name="const", bufs=1))
        big = ctx.enter_context(tc.tile_pool(name="big", bufs=1))
        prb = ctx.enter_context(tc.tile_pool(name="prb", bufs=2))
        stage = ctx.enter_context(tc.tile_pool(name="stage", bufs=3))
        norm = ctx.enter_context(tc.tile_pool(name="norm", bufs=2))
        ypool = ctx.enter_context(tc.tile_pool(name="ypool", bufs=3))
        psA = ctx.enter_context(tc.tile_pool(name="psA", bufs=3, space="PSUM"))
        psB = ctx.enter_context(tc.tile_pool(name="psB", bufs=2, space="PSUM"))

        # ---- constant loads
        xT = const.tile([128, 6, N], bf)
        nc.sync.dma_start(out=xT, in_=xT_d.rearrange("(s p) n -> p s n", p=128))
        wq = const.tile([128, 6, C], bf)
        nc.sync.dma_start(out=wq, in_=wq_d.rearrange("(s p) m -> p s m", p=128))
        wk = const.tile([128, 6, C], bf)
        nc.sync.dma_start(out=wk, in_=wk_d.rearrange("(s p) m -> p s m", p=128))
        wv = const.tile([128, 6, C], bf)
        nc.sync.dma_start(out=wv, in_=wv_d.rearrange("(s p) m -> p s m", p=128))
        wp = const.tile([128, 6, C], bf)
        nc.sync.dma_start(out=wp, in_=wp_d.rearrange("(s p) m -> p s m", p=128))
        bqt = const.tile([128, 6], f32)
        nc.sync.dma_start(out=bqt, in_=bqt_d[:, :])
        bkt = const.tile([128, 6], f32)
        nc.sync.dma_start(out=bkt, in_=bkt_d[:, :])
        bv = const.tile([1, C], bf)
        nc.sync.dma_start(out=bv, in_=bv_d[:, :])
        bp = const.tile([1, C], bf)
        nc.sync.dma_start(out=bp, in_=bp_d[:, :])
        ones512 = const.tile([1, 512], bf)
        nc.vector.memset(ones512, 1.0)
        ones128 = const.tile([128, 1], bf)
        nc.vector.memset(ones128, 1.0)

        # ---- persistent intermediates
        qT = big.tile([128, 6, N], bf)      # q*scale+bq, [32h+d -> (p,s)], n
        kT = big.tile([128, 6, N], bf)
        v = big.tile([128, 8, H, DH], bf)   # [token%128, token//128, h, d]
        outT = big.tile([128, 6, N], bf)    # unnorm attn out.T [32h+d, n]

        # ---- qkv projections
        def emit_qk_group(j):
            for (wt, bt, dstT) in ((wq, bqt, qT), (wk, bkt, kT)):
                for t in range(2):
                    ps = psA.tile([128, 2, 512], f32, tag="ps")
                    pq = ps[:, 0, :]
                    for s in range(6):
                        nc.tensor.matmul(
                            pq,
                            lhsT=wt[:, s, 128 * j:128 * (j + 1)],
                            rhs=xT[:, s, 512 * t:512 * (t + 1)],
                            start=(s == 0), stop=(s == 5))
                    nc.vector.tensor_scalar(
                        out=dstT[:, j, 512 * t:512 * (t + 1)], in0=pq,
                        scalar1=bt[:, j:j + 1], scalar2=None, op0=Add)

        def emit_v():
            for i in range(8):
                for (f0, fw) in ((0, 512), (512, 256)):
                    ps = psA.tile([128, 2, 512], f32, tag="ps")
                    pv_ = ps[:, 0, :fw]
                    for s in range(6):
                        nc.tensor.matmul(
                            pv_,
                            lhsT=xT[:, s, 128 * i:128 * (i + 1)],
                            rhs=wv[:, s, f0:f0 + fw],
                            start=(s == 0), stop=False)
                    nc.tensor.matmul(
                        pv_, lhsT=ones512[:, :128], rhs=bv[:, f0:f0 + fw],
                        start=False, stop=True)
                    nc.vector.tensor_copy(
                        out=v[:, i, f0 // DH:(f0 + fw) // DH, :],
                        in_=pv_.rearrange("p (h d) -> p h d", d=DH))

        def emit_proj_chunk(i):
            ytile = ypool.tile([128, C], f32, tag="y")
            for (f0, fw) in ((0, 512), (512, 256)):
                ps = psA.tile([128, 2, 512], f32, tag="ps")
                py = ps[:, 0, :fw]
                for s in range(6):
                    nc.tensor.matmul(
                        py,
                        lhsT=outT[:, s, 128 * i:128 * (i + 1)],
                        rhs=wp[:, s, f0:f0 + fw],
                        start=(s == 0), stop=False)
                nc.tensor.matmul(
                    py, lhsT=ones512[:, :128], rhs=bp[:, f0:f0 + fw],
                    start=False, stop=True)
                nc.vector.tensor_copy(out=ytile[:, f0:f0 + fw], in_=py)
            nc.sync.dma_start(out=out_d[128 * i:128 * (i + 1), :], in_=ytile)

        # ---- attention for one (g, qt): 16-tile scores, exp, *expb, PV, norm
        def emit_attention(g, qt):
            qs = slice(512 * qt, 512 * (qt + 1))
            # probs layout [p, kt, hl, q] so each kt slice is N=2048-contig
            probs = prb.tile([128, 8, 4, 512], bf, tag="probs")
            for kt in range(8):
                eb = stage.tile([128, 4, 512], bf, tag="eb")
                nc.sync.dma_start(
                    out=eb, in_=eb_d[g, 128 * kt:128 * (kt + 1), :, qs])
                scA = psA.tile([128, 2, 512], f32, tag="ps")
                scB = psA.tile([128, 2, 512], f32, tag="ps")
                # 16-way packing: head i in (row 32i), k-chunk j -> (col 32j)
                for i in range(4):
                    sc = scA if i < 2 else scB
                    for j in range(4):
                        nc.tensor.matmul(
                            sc[32 * j:32 * (j + 1), i % 2, :],
                            lhsT=kT[32 * i:32 * (i + 1), g,
                                    128 * kt + 32 * j:128 * kt + 32 * (j + 1)],
                            rhs=qT[32 * i:32 * (i + 1), g, qs],
                            start=True, stop=True,
                            tile_position=(32 * i, 32 * j),
                            skip_group_check=True)
                nc.scalar.activation(out=probs[:, kt, 0:2, :], in_=scA, func=Exp)
                nc.scalar.activation(out=probs[:, kt, 2:4, :], in_=scB, func=Exp)
                nc.vector.tensor_mul(
                    out=probs[:, kt, :, :], in0=probs[:, kt, :, :], in1=eb)
            # PV + denominators (col-tiled, M=32 v + M=1 ones)
            pv = psB.tile([128, 512], f32, tag="pv")
            dn = psB.tile([128, 512], f32, tag="pv")
            nc.vector.memset(dn, 1.0)
            for kt in range(8):
                for hl in range(4):
                    nc.tensor.matmul(
                        pv[32 * hl:32 * (hl + 1), :],
                        lhsT=v[:, kt, 4 * g + hl, :],
                        rhs=probs[:, kt, hl, :],
                        start=(kt == 0), stop=(kt == 7),
                        tile_position=(0, 32 * hl),
                        skip_group_check=True)
                    nc.tensor.matmul(
                        dn[32 * hl:32 * hl + 1, :],
                        lhsT=ones128[:, :],
                        rhs=probs[:, kt, hl, :],
                        start=(kt == 0), stop=(kt == 7),
                        tile_position=(0, 32 * hl),
                        skip_group_check=True)
            nc.vector.tensor_copy(out=outT[:, g, qs], in_=pv)
            # normalization: approx-recip (garbage rows are 1.0) -> DRAM ->
            # partition-broadcast -> multiply
            slot = 6 * qt + g
            rec = norm.tile([128, 512], f32, tag="rec")
            dtile = norm.tile([128, 512], bf, tag="den")
            rtile = norm.tile([128, 512], bf, tag="rb")
            nc.vector.reciprocal_approx_fast(out=rec, in_=dn)
            nc.vector.tensor_copy(out=dtile, in_=rec)
            nc.sync.dma_start(out=scr_d[:, slot, :], in_=dtile)
            for hl in range(4):
                nc.sync.dma_start(
                    out=rtile[32 * hl:32 * (hl + 1), :],
                    in_=scr_d[32 * hl:32 * hl + 1, slot, :].to_broadcast(
                        (32, 512)))
            nc.vector.tensor_mul(
                out=outT[:, g, qs], in0=outT[:, g, qs], in1=rtile)

        # ---- schedule: qk0/v first so attention starts early; remaining qk
        # groups and proj chunks interleave into PE idle time during attention
        emit_qk_group(0)
        emit_v()
        emit_qk_group(1)
        for qt in range(2):
            for g in range(6):
                emit_attention(g, qt)
                if qt == 0 and g < 4:
                    emit_qk_group(g + 2)
            for i in range(4 * qt, 4 * qt + 4):
                emit_proj_chunk(i)

    nc.finalize()
    _CACHE["nc"] = nc
    return nc


def _prep_shared(shared_rel_pos, Wqkv, bqkv, Wproj, bproj):
    """Host-side weight rearrangement shared by all cores (float32 in)."""
    w3 = np.asarray(Wqkv, np.float32).reshape(H, 3, DH, C)
    wq_t = (w3[:, 0] * SCALE).transpose(2, 0, 1).reshape(C, C)
    wk_t = w3[:, 1].transpose(2, 0, 1).reshape(C, C)
    wv_t = w3[:, 2].transpose(2, 0, 1).reshape(C, C)
    b3 = np.asarray(bqkv, np.float32).reshape(H, 3, DH)
    bq_a = (b3[:, 0] * SCALE).reshape(C)
    bk_a = b3[:, 1].reshape(C)
    bv_a = b3[:, 2].reshape(1, C)
    # exp(rel)^T grouped: [g, k, hl, q]
    expb = np.exp(np.asarray(shared_rel_pos, np.float32))
    expb = expb.transpose(0, 2, 1).reshape(HG, 4, N, N).transpose(0, 2, 1, 3)
    wp_t = np.asarray(Wproj, np.float32).T.copy()
    bp_a = np.asarray(bproj, np.float32).reshape(1, C)
    return {
        "wq": np.ascontiguousarray(wq_t).astype(BF16),
        "wk": np.ascontiguousarray(wk_t).astype(BF16),
        "wv": np.ascontiguousarray(wv_t).astype(BF16),
        "bqt": np.ascontiguousarray(bq_a.reshape(6, 128).T),
        "bkt": np.ascontiguousarray(bk_a.reshape(6, 128).T),
        "bv": bv_a.astype(BF16),
        "expb": np.ascontiguousarray(expb).astype(BF16),
        "wpj": wp_t.astype(BF16),
        "bpj": bp_a.astype(BF16),
    }


def _in_maps(x, shared):
    x = np.asarray(x, np.float32)
    maps = []
    for b in range(B):
        m = dict(shared)
        m["xT"] = np.ascontiguousarray(x[b].T).astype(BF16)
        maps.append(m)
    return maps


def kernel(**inputs):
    from concourse.bass_utils import run_bass_kernel_spmd

    nc = _build()
    shared = _prep_shared(
        inputs["shared_rel_pos"], inputs["Wqkv"], inputs["bqkv"],
        inputs["Wproj"], inputs["bproj"])
    maps = _in_maps(inputs["x"], shared)
    res = run_bass_kernel_spmd(nc, maps, core_ids=list(range(B)))
    out = np.stack([np.asarray(res.results[i]["out"], np.float32)
                    for i in range(B)])
    return out
